# revision 7
# baseline (speedup 1.0000x reference)
"""CANModule forward kernel for 8 Trainium2 NeuronCores.

The reference computes
    new_place = relu(place_cells + ec @ W_ec + sum_i grid_i @ W_mec_i)
(the MEC grid updates are computed-then-deleted in the reference - dead
code - so W_gh*/W_gg* never need to reach the device).

Strategy: shard the HPC output dim (8192) column-wise across 8 cores
(1024 cols each).  Weights are quantized host-side to fp8 E3M4 (x32 so
the sigma~1/64..1/32 gaussians land in e3m4's normal range), halving
HBM traffic vs fp16 - the kernel is memory-bound.  The 1/32 descale is
folded into A on the host.

Per core the matmul runs W-STATIONARY: for each 128-wide tile t of the
1024 output cols and each of 88 K-chunks,
    psum_t[128, 4] += W[k-chunk, t-tile][128, 128].T-as-lhsT @ A_k[128, 4]
so W is ingested through LDWEIGHTS with FastWeightLoad instead of
streaming as the moving operand.  PE sustains ~42.7ns/chunk (dispatch
bound at N=4), i.e. ~30.1us for the 704-chunk stream.

Timeline learnings (ntff traces):
 - exec_time = last engine's final instruction end (incl. the ~7.2us
   runtime-injected epilogue that zeroes the 256-sem file per engine)
   minus the first non-ceremony instruction start.  The runtime preamble
   (~6.5us barrier+TENSOR_LOAD) is excluded; the epilogue is not, and is
   outside NEFF control.
 - The W stream runs at ~420-450 GB/s aggregate; per-SDMA-engine rate is
   ~25.5 GB/s and each of the 16 engines owns a fixed 1/16 of every
   transfer (8 partitions each).  Engine E64 additionally carries the
   ~87KB PE instruction paging and E79 is intrinsically ~15% slower, so
   piece-completion sems trail the fast engines by up to ~6us.
 - A/pl must land before the first matmul: ship them as ONE small f16
   tensor FIRST on the same HWDGE ring as W (FIFO => landed before
   piece 0's sem), with a small ramp-up piece so the PE starts ~9us.
 - relu (+pl bias) and the output store both live on the Scalar engine:
   ACT is in-order, so the final-tile relu -> store chain carries no
   cross-engine hops; pl rides the a tensor as raw f32 bytes viewed f16
   (bitcast back on-device - exact).
"""

import numpy as np
import ml_dtypes

import concourse.bass as bass
import concourse.mybir as mybir
import concourse.tile as tile
from concourse.bass_utils import run_bass_kernel_spmd

N_CORES = 8
B = 4
EC = 4096
MECS = (1024, 2048, 4096)
HPC = 8192
SHARD = HPC // N_CORES          # 1024 output cols per core
K_TOTAL = EC + sum(MECS)        # 11264 contraction rows
P = 128
KC = K_TOTAL // P               # 88 K-chunks
T = SHARD // P                  # 8 col-tiles of 128 output cols
W_SCALE = 32.0                  # e3m4 range fit; 1/32 folded into A
ACOLS = KC * B + 2 * T          # A.T (chunk-major) + pl as f32-bytes-in-f16

CONFIG = {
    "trace": False,
    "strip_ceremony": True,
    # W DMA schedule: (tile, chunks).  One HWDGE ring (sync), FIFO.  The
    # `a` tensor goes first (PE gate + FIFO => landed before piece 0).
    # Coarse pieces: every piece costs ~128 descriptors on the (engine-
    # bottlenecked) SDMA queues regardless of width, and the PE tracks the
    # piece-sem pace anyway, so a start ramp only adds descriptor overhead
    # (measured +0.7us on the straggler engine).  Keep a small taper at
    # the end so little compute remains after the last piece sem.
    # Tail: the straggler SDMA engine (it also carries ~87KB of PE
    # instruction paging) delivers the last pieces' shares back-to-back
    # ~5us after the pack, and the PE bursts at ~27ns/chunk once a sem
    # fires - so taper geometrically: each piece's chunks finish just as
    # the next (straggler-paced) sem lands, instead of 88 chunks queueing
    # behind one late sem.
    "schedule": (
        (0, 88), (1, 88), (2, 88), (3, 88), (4, 88), (5, 88), (6, 88),
        (7, 40), (7, 24), (7, 12), (7, 6), (7, 4), (7, 2),
    ),
}
_CACHE = {}


def _build():
    DT_A = mybir.dt.float16
    SCHED = CONFIG["schedule"]

    nc = bass.Bass()
    a = nc.dram_tensor("a", [P, ACOLS], DT_A, kind="ExternalInput")
    w = nc.dram_tensor("w", [T * P, KC * P], mybir.dt.float8e3, kind="ExternalInput")
    out = nc.dram_tensor("out", [P, T * B], mybir.dt.float32, kind="ExternalOutput")

    with tile.TileContext(nc) as tc:
        with (
            tc.tile_pool(name="const", bufs=1) as cpool,
            tc.tile_pool(name="wload", bufs=1) as wpool,
            tc.tile_pool(name="outp", bufs=1) as opool,
            tc.tile_pool(name="acc", bufs=1, space="PSUM") as pspool,
        ):
            a_t = cpool.tile([P, ACOLS], DT_A)
            warm_t = cpool.tile([P, P], DT_A)
            o_t = opool.tile([P, T * B], mybir.dt.float32)
            gate_t = opool.tile([P, 1], mybir.dt.float32)

            nc.vector.memset(warm_t[:], 0.0)

            ps_tiles = []
            for t in range(T):
                pst = pspool.tile([P, B + B], mybir.dt.float32, name=f"ps{t}")
                ps_tiles.append(pst)

            # a (activations + pl bytes) first on the sync HWDGE ring; the
            # W pieces follow in FIFO order on the same ring.
            nc.sync.dma_start(a_t[:], a[:])
            w_r = w.rearrange("(t p) m -> t p m", p=P)
            w_tiles = []
            k_off = [0] * T
            for i, (t, nkc) in enumerate(SCHED):
                k0 = k_off[t]
                k1 = k0 + nkc
                k_off[t] = k1
                wt = wpool.tile([P, nkc * P], mybir.dt.float8e3, name=f"w{t}_{i}")
                nc.sync.dma_start(wt[:], w_r[t][:, k0 * P : k1 * P])
                w_tiles.append((t, wt, k0, k1))
            assert k_off == [KC] * T

            # PE gate: absorbs the a-DMA wait so each main-loop matmul
            # carries at most one sem wait (its W-piece DMA).
            nc.tensor.matmul(
                ps_tiles[-1][:, B : 2 * B],
                warm_t[:, 0:P],
                a_t[:, 0:B],
                start=True,
                stop=True,
            )
            # ACT gate: absorbs the a-DMA wait on the (in-order) scalar
            # stream so each relu below carries only its PE wait.
            nc.scalar.copy(gate_t[:], a_t[:, 0:1])

            for t, wt, k0, k1 in w_tiles:
                ps = ps_tiles[t][:, 0:B]
                for k in range(k0, k1):
                    c = (k - k0) * P
                    nc.tensor.matmul(
                        ps,
                        wt[:, c : c + P],
                        a_t[:, B * k : B * (k + 1)],
                        start=(k == 0),
                        stop=(k == KC - 1),
                    )
                if k1 == KC:
                    bias = a_t[:, KC * B + 2 * t : KC * B + 2 * t + 2].bitcast(
                        mybir.dt.float32
                    )
                    nc.scalar.activation(
                        o_t[:, B * t : B * (t + 1)],
                        ps,
                        mybir.ActivationFunctionType.Relu,
                        bias=bias,
                        scale=1.0,
                    )
            nc.scalar.dma_start(out[:], o_t[:])

    _strip_redundant_waits(nc)
    _unhook_store_completion(nc)
    _move_store_post_barrier(nc)
    if CONFIG["strip_ceremony"]:
        _strip_ceremony(nc)
    return nc


def _move_store_post_barrier(nc):
    """Run the output store's descriptor generation during the epilogue.

    The runtime epilogue (whose ~6us sem-zeroing dominates the kernel
    tail) starts only after an all-engine butterfly; the last arriver is
    ACT, which was held by the store DMA's ~0.7us DGE.  Move the store
    instruction after ACT's end-of-kernel barrier arrive/release pair so
    ACT arrives right after the last relu; the store (whose completion is
    already off-path via sem 0) DGEs and lands during the zeroing.  Its
    `Activation >= 9` wait is implied by ACT program order and must be
    dropped: the Pool RANGE_CLEAR zeroes the Activation sem concurrently.
    """
    blocks = nc.m.functions[0].blocks
    body, end = blocks[1], blocks[-1]
    idx = [
        n for n, i in enumerate(body.instructions)
        if type(i).__name__ == "InstDMACopy" and i.engine == mybir.EngineType.Activation
    ]
    assert len(idx) == 1
    store = body.instructions.pop(idx[-1])
    assert store.sync_info.on_update[0].ant_name == "store_sink"
    store.sync_info.on_wait = []
    act_rel = [
        n for n, i in enumerate(end.instructions)
        if type(i).__name__ == "InstEventSemaphore"
        and i.engine == mybir.EngineType.Activation
    ]
    # the end block carries the all-engine barrier twice ("just to be
    # safe" in bass.reset); _strip_ceremony drops the second copy along
    # with everything after the InstISA.  Insert after the first release.
    assert act_rel, act_rel
    end.instructions.insert(act_rel[0] + 1, store)


def _unhook_store_completion(nc):
    """Take the output store's completion off the end-of-kernel critical
    path.

    The runtime-injected epilogue (sem-file zeroing, ~6us) starts only
    after an all-engine butterfly whose last arriver is SP, held by a
    drain for the store's HBM completion (~1.5us of DGE + write-ack
    latency).  The store physically lands during the epilogue with huge
    margin, so: retarget the store's completion increment to sem 0 (never
    zeroed by the runtime, never waited on - repeated executions just
    accumulate a benign count there, and the Tile RANGE_CLEAR can no
    longer race a late increment on a DMAHW lane), and delete the drain
    that waited for it.
    """
    insts = [i for blk in nc.m.functions[0].blocks for i in blk.instructions]
    store = [i for i in insts if type(i).__name__ == "InstDMACopy"][-1]
    si = store.sync_info
    assert si and len(si.on_update) == 1
    lane = si.on_update[0].ant_name
    upd = si.on_update[0]
    upd.id = 0
    upd.ant_name = "store_sink"
    names = nc.m.ant_sem_names
    names[0] = ["store_sink"]

    cum = 0
    for i in insts:
        if i.sync_info and i is not store:
            cum += sum(
                u.update_value for u in i.sync_info.on_update if u.ant_name == lane
            )
    end = nc.m.functions[0].blocks[-1]
    drop = []
    for n, i in enumerate(end.instructions):
        if type(i).__name__ != "InstDrain" or not i.sync_info:
            continue
        w = [x for x in i.sync_info.on_wait if x.ant_name == lane]
        if not w:
            continue
        assert w[0].wait_value == cum + 16, (w, cum)
        drop.append(n)
    assert len(drop) == 1, f"expected one store-lane drain, got {drop}"
    del end.instructions[drop[0]]


def _strip_redundant_waits(nc):
    """The DMA / Matmult / Activation pseudo-ops encode a single sync wait,
    but Tile can emit more.

    1. The output-store DMA may get {Activation >= n, DMAHW_k >= m}.  The
       Activation wait (or the in-order ACT stream itself) implies the DMA
       wait transitively: ACT is in-order and every relu is gated on PE
       progress whose matmuls waited on that W DMA.
    2. The end-of-kernel quiesce drain waits on every proc lane, but the
       kernel is one dependency chain ending in the output-store DMA:
       "store landed" implies everything else.
    """
    insts = [i for blk in nc.m.functions[0].blocks for i in blk.instructions]
    for inst in insts:
        ty = type(inst).__name__
        si = inst.sync_info
        if si is None or len(si.on_wait) <= 1:
            continue
        if ty == "InstDMACopy":
            waits = list(si.on_wait)
            engine = [
                w
                for w in waits
                if w.ant_name.split("_")[0] in ("PE", "Activation", "DVE", "Pool", "SP")
            ]
            rest = [w for w in waits if w not in engine]
            dma_lanes = [w for w in rest if w.ant_name.startswith("DMA")]
            if len(engine) == 1 and len(dma_lanes) == len(rest):
                si.on_wait = engine
                continue
        if ty in ("InstDMACopy", "InstMatmult", "InstActivation"):
            raise RuntimeError(
                f"{inst.name} ({ty}) still has {len(si.on_wait)} waits: {si}"
            )

    store = [i for i in insts if type(i).__name__ == "InstDMACopy"][-1]
    assert store.sync_info and len(store.sync_info.on_update) == 1
    lane = store.sync_info.on_update[0].ant_name
    cum = 0
    for i in insts:
        if i.sync_info:
            cum += sum(
                u.update_value for u in i.sync_info.on_update if u.ant_name == lane
            )
    for inst in insts:
        if type(inst).__name__ != "InstDrain":
            continue
        si = inst.sync_info
        if si is None or len(si.on_wait) <= 1:
            continue
        keep = [w for w in si.on_wait if w.ant_name == lane and w.wait_value == cum]
        assert keep, f"drain {inst.name} lacks the store-lane wait (cum={cum}): {si}"
        si.on_wait = keep[:1]


def _strip_ceremony(nc):
    """Remove the all-engine butterfly barriers that bracket the kernel.

    Every data dependency is carried by absolute-valued semaphore waits from
    a zeroed sem file, so engines may enter their streams unaligned.
    """
    blocks = nc.m.functions[0].blocks
    b0 = blocks[0]
    drop = [
        n
        for n, i in enumerate(b0.instructions)
        if type(i).__name__ in ("InstDrain", "InstEventSemaphore")
    ]
    for n in reversed(drop):
        del b0.instructions[n]

    end = blocks[-1]
    isa_idx = [
        n for n, i in enumerate(end.instructions) if type(i).__name__ == "InstISA"
    ]
    if isa_idx:
        for n in range(len(end.instructions) - 1, isa_idx[-1], -1):
            del end.instructions[n]


def kernel(**inputs):
    ec = np.asarray(inputs["ec_activations"], dtype=np.float32)
    place = np.asarray(inputs["place_cells"], dtype=np.float32)
    grids = [np.asarray(inputs[f"grid{i}"], dtype=np.float32) for i in range(3)]
    W_ec = np.asarray(inputs["W_ec"], dtype=np.float32)
    W_mec = [np.asarray(inputs[f"W_mec{i}"], dtype=np.float32) for i in range(3)]

    X = np.concatenate(grids, axis=1)                                   # [1, 7168]
    A = np.concatenate([ec, np.broadcast_to(X, (B, X.shape[1]))], 1)    # [4, 11264]
    A = A / W_SCALE              # fold the W descale into A
    # swizzle A.T into [p, (k b)] chunk-major layout
    aT_sw = np.ascontiguousarray(
        A.T.reshape(KC, P, B).transpose(1, 0, 2)
    ).reshape(P, KC * B).astype(np.float16)

    W_all = np.concatenate([W_ec] + W_mec, axis=0)                      # [11264, 8192]
    Wq = (W_all * W_SCALE).astype(ml_dtypes.float8_e3m4)

    key = "nc_v2_" + str(CONFIG["strip_ceremony"])
    nc = _CACHE.get(key)
    if nc is None:
        nc = _CACHE[key] = _build()

    in_maps = []
    for c in range(N_CORES):
        cols = slice(SHARD * c, SHARD * (c + 1))
        # [t*128+p, k*128+j] = Wq[k*128+p, t*128+j]
        w_sw = np.ascontiguousarray(
            Wq[:, cols].reshape(KC, P, T, P).transpose(2, 1, 0, 3)
        ).reshape(T * P, KC * P)
        pl_sw = np.ascontiguousarray(
            place[0, cols].reshape(T, P).T
        ).astype(np.float32)                                            # [128, 8]
        a_host = np.concatenate([aT_sw, pl_sw.view(np.float16)], axis=1)
        in_maps.append({"a": a_host, "w": w_sw})
    res = run_bass_kernel_spmd(
        nc, in_maps, core_ids=list(range(N_CORES)), trace=CONFIG["trace"]
    )
    _CACHE["last_results"] = res
    outs = []
    for c in range(N_CORES):
        o = np.asarray(res.results[c]["out"])                           # [128, 8*4]
        outs.append(o.reshape(P, T, B).transpose(2, 1, 0).reshape(B, SHARD))
    return np.concatenate(outs, axis=1)


# revision 9
# speedup vs baseline: 1.0257x; 1.0257x over previous
"""CANModule forward kernel for 8 Trainium2 NeuronCores.

The reference computes
    new_place = relu(place_cells + ec @ W_ec + sum_i grid_i @ W_mec_i)
(the MEC grid updates are computed-then-deleted in the reference - dead
code - so W_gh*/W_gg* never need to reach the device).

Strategy: shard the HPC output dim (8192) column-wise across 8 cores
(1024 cols each).  Weights are quantized host-side to fp8 E3M4 (x32 so
the sigma~1/64..1/32 gaussians land in e3m4's normal range), halving
HBM traffic vs fp16 - the kernel is memory-bound.  The 1/32 descale is
folded into A on the host.

Per core the matmul runs W-STATIONARY: for each 128-wide tile t of the
1024 output cols and each of 88 K-chunks,
    psum_t[128, 4] += W[k-chunk, t-tile][128, 128].T-as-lhsT @ A_k[128, 4]
so W is ingested through LDWEIGHTS with FastWeightLoad instead of
streaming as the moving operand.  PE sustains ~42.7ns/chunk (dispatch
bound at N=4), i.e. ~30.1us for the 704-chunk stream.

Timeline learnings (ntff traces):
 - exec_time = last engine's final instruction end (incl. the ~7.2us
   runtime-injected epilogue that zeroes the 256-sem file per engine)
   minus the first non-ceremony instruction start.  The runtime preamble
   (~6.5us barrier+TENSOR_LOAD) is excluded; the epilogue is not, and is
   outside NEFF control.
 - The W stream runs at ~420-450 GB/s aggregate; per-SDMA-engine rate is
   ~25.5 GB/s and each of the 16 engines owns a fixed 1/16 of every
   transfer (8 partitions each).  Engine E64 additionally carries the
   ~87KB PE instruction paging and E79 is intrinsically ~15% slower, so
   piece-completion sems trail the fast engines by up to ~6us.
 - A/pl must land before the first matmul: ship them as ONE small f16
   tensor FIRST on the same HWDGE ring as W (FIFO => landed before
   piece 0's sem), with a small ramp-up piece so the PE starts ~9us.
 - relu (+pl bias) and the output store both live on the Scalar engine:
   ACT is in-order, so the final-tile relu -> store chain carries no
   cross-engine hops; pl rides the a tensor as raw f32 bytes viewed f16
   (bitcast back on-device - exact).
"""

import numpy as np
import ml_dtypes

import concourse.bass as bass
import concourse.mybir as mybir
import concourse.tile as tile
from concourse.bass_utils import run_bass_kernel_spmd

N_CORES = 8
B = 4
EC = 4096
MECS = (1024, 2048, 4096)
HPC = 8192
SHARD = HPC // N_CORES          # 1024 output cols per core
K_TOTAL = EC + sum(MECS)        # 11264 contraction rows
P = 128
KC = K_TOTAL // P               # 88 K-chunks
T = SHARD // P                  # 8 col-tiles of 128 output cols
W_SCALE = 32.0                  # e3m4 range fit; 1/32 folded into A
ACOLS = KC * B + 2 * T          # A.T (chunk-major) + pl as f32-bytes-in-f16

CONFIG = {
    "trace": False,
    "strip_ceremony": True,
    # W DMA schedule: (tile, chunks).  One HWDGE ring (sync), FIFO.  The
    # `a` tensor goes first (PE gate + FIFO => landed before piece 0).
    # Coarse pieces: every piece costs ~128 descriptors on the (engine-
    # bottlenecked) SDMA queues regardless of width, and the PE tracks the
    # piece-sem pace anyway, so a start ramp only adds descriptor overhead
    # (measured +0.7us on the straggler engine).  Keep a small taper at
    # the end so little compute remains after the last piece sem.
    # Tail: the straggler SDMA engine (it also carries ~87KB of PE
    # instruction paging) delivers the last pieces' shares back-to-back
    # ~5us after the pack, and the PE bursts at ~27ns/chunk once a sem
    # fires - so taper geometrically: each piece's chunks finish just as
    # the next (straggler-paced) sem lands, instead of 88 chunks queueing
    # behind one late sem.
    # Piece sems are straggler-paced (~44ns per chunk-share on the slow
    # SDMA engines) while the PE bursts at ~27.5ns/chunk, so the tail cost
    # is max_k [sem_k + 27.5ns x chunks_after_k].  A geometric taper over
    # the last TWO tiles keeps that bound ~0.4us past the straggler's last
    # byte (a single 88-chunk final tile leaves 176 chunks stranded behind
    # a late sem: +2.4us).
    "schedule": (
        (0, 88), (1, 88), (2, 88), (3, 88), (4, 88), (5, 88),
        (6, 48), (6, 40),
        (7, 24), (7, 20), (7, 16), (7, 12), (7, 8), (7, 6), (7, 2),
    ),
}
_CACHE = {}


def _build():
    DT_A = mybir.dt.float16
    SCHED = CONFIG["schedule"]

    nc = bass.Bass()
    a = nc.dram_tensor("a", [P, ACOLS], DT_A, kind="ExternalInput")
    w = nc.dram_tensor("w", [T * P, KC * P], mybir.dt.float8e3, kind="ExternalInput")
    out = nc.dram_tensor("out", [P, T * B], mybir.dt.float32, kind="ExternalOutput")

    with tile.TileContext(nc) as tc:
        with (
            tc.tile_pool(name="const", bufs=1) as cpool,
            tc.tile_pool(name="wload", bufs=1) as wpool,
            tc.tile_pool(name="outp", bufs=1) as opool,
            tc.tile_pool(name="acc", bufs=1, space="PSUM") as pspool,
        ):
            a_t = cpool.tile([P, ACOLS], DT_A)
            warm_t = cpool.tile([P, P], DT_A)
            o_t = opool.tile([P, T * B], mybir.dt.float32)
            gate_t = opool.tile([P, 1], mybir.dt.float32)

            nc.vector.memset(warm_t[:], 0.0)

            ps_tiles = []
            for t in range(T):
                pst = pspool.tile([P, B + B], mybir.dt.float32, name=f"ps{t}")
                ps_tiles.append(pst)

            # a (activations + pl bytes) first on the sync HWDGE ring; the
            # W pieces follow in FIFO order on the same ring.
            nc.sync.dma_start(a_t[:], a[:])
            w_r = w.rearrange("(t p) m -> t p m", p=P)
            w_tiles = []
            k_off = [0] * T
            for i, (t, nkc) in enumerate(SCHED):
                k0 = k_off[t]
                k1 = k0 + nkc
                k_off[t] = k1
                wt = wpool.tile([P, nkc * P], mybir.dt.float8e3, name=f"w{t}_{i}")
                nc.sync.dma_start(wt[:], w_r[t][:, k0 * P : k1 * P])
                w_tiles.append((t, wt, k0, k1))
            assert k_off == [KC] * T

            # PE gate: absorbs the a-DMA wait so each main-loop matmul
            # carries at most one sem wait (its W-piece DMA).
            nc.tensor.matmul(
                ps_tiles[-1][:, B : 2 * B],
                warm_t[:, 0:P],
                a_t[:, 0:B],
                start=True,
                stop=True,
            )
            # ACT gate: absorbs the a-DMA wait on the (in-order) scalar
            # stream so each relu below carries only its PE wait.
            nc.scalar.copy(gate_t[:], a_t[:, 0:1])

            for t, wt, k0, k1 in w_tiles:
                ps = ps_tiles[t][:, 0:B]
                for k in range(k0, k1):
                    c = (k - k0) * P
                    nc.tensor.matmul(
                        ps,
                        wt[:, c : c + P],
                        a_t[:, B * k : B * (k + 1)],
                        start=(k == 0),
                        stop=(k == KC - 1),
                    )
                if k1 == KC:
                    bias = a_t[:, KC * B + 2 * t : KC * B + 2 * t + 2].bitcast(
                        mybir.dt.float32
                    )
                    nc.scalar.activation(
                        o_t[:, B * t : B * (t + 1)],
                        ps,
                        mybir.ActivationFunctionType.Relu,
                        bias=bias,
                        scale=1.0,
                    )
            nc.scalar.dma_start(out[:], o_t[:])

    _strip_redundant_waits(nc)
    _unhook_store_completion(nc)
    _move_store_post_barrier(nc)
    if CONFIG["strip_ceremony"]:
        _strip_ceremony(nc)
    return nc


def _move_store_post_barrier(nc):
    """Run the output store's descriptor generation during the epilogue.

    The runtime epilogue (whose ~6us sem-zeroing dominates the kernel
    tail) starts only after an all-engine butterfly; the last arriver is
    ACT, which was held by the store DMA's ~0.7us DGE.  Move the store
    instruction after ACT's end-of-kernel barrier arrive/release pair so
    ACT arrives right after the last relu; the store (whose completion is
    already off-path via sem 0) DGEs and lands during the zeroing.  Its
    `Activation >= 9` wait is implied by ACT program order and must be
    dropped: the Pool RANGE_CLEAR zeroes the Activation sem concurrently.
    """
    blocks = nc.m.functions[0].blocks
    body, end = blocks[1], blocks[-1]
    idx = [
        n for n, i in enumerate(body.instructions)
        if type(i).__name__ == "InstDMACopy" and i.engine == mybir.EngineType.Activation
    ]
    assert len(idx) == 1
    store = body.instructions.pop(idx[-1])
    assert store.sync_info.on_update[0].ant_name == "store_sink"
    store.sync_info.on_wait = []
    act_rel = [
        n for n, i in enumerate(end.instructions)
        if type(i).__name__ == "InstEventSemaphore"
        and i.engine == mybir.EngineType.Activation
    ]
    # the end block carries the all-engine barrier twice ("just to be
    # safe" in bass.reset); _strip_ceremony drops the second copy along
    # with everything after the InstISA.  Insert after the first release.
    assert act_rel, act_rel
    end.instructions.insert(act_rel[0] + 1, store)


def _unhook_store_completion(nc):
    """Take the output store's completion off the end-of-kernel critical
    path.

    The runtime-injected epilogue (sem-file zeroing, ~6us) starts only
    after an all-engine butterfly whose last arriver is SP, held by a
    drain for the store's HBM completion (~1.5us of DGE + write-ack
    latency).  The store physically lands during the epilogue with huge
    margin, so: retarget the store's completion increment to sem 0 (never
    zeroed by the runtime, never waited on - repeated executions just
    accumulate a benign count there, and the Tile RANGE_CLEAR can no
    longer race a late increment on a DMAHW lane), and delete the drain
    that waited for it.
    """
    insts = [i for blk in nc.m.functions[0].blocks for i in blk.instructions]
    store = [i for i in insts if type(i).__name__ == "InstDMACopy"][-1]
    si = store.sync_info
    assert si and len(si.on_update) == 1
    lane = si.on_update[0].ant_name
    upd = si.on_update[0]
    upd.id = 0
    upd.ant_name = "store_sink"
    names = nc.m.ant_sem_names
    names[0] = ["store_sink"]

    cum = 0
    for i in insts:
        if i.sync_info and i is not store:
            cum += sum(
                u.update_value for u in i.sync_info.on_update if u.ant_name == lane
            )
    end = nc.m.functions[0].blocks[-1]
    drop = []
    for n, i in enumerate(end.instructions):
        if type(i).__name__ != "InstDrain" or not i.sync_info:
            continue
        w = [x for x in i.sync_info.on_wait if x.ant_name == lane]
        if not w:
            continue
        assert w[0].wait_value == cum + 16, (w, cum)
        drop.append(n)
    assert len(drop) == 1, f"expected one store-lane drain, got {drop}"
    del end.instructions[drop[0]]


def _strip_redundant_waits(nc):
    """The DMA / Matmult / Activation pseudo-ops encode a single sync wait,
    but Tile can emit more.

    1. The output-store DMA may get {Activation >= n, DMAHW_k >= m}.  The
       Activation wait (or the in-order ACT stream itself) implies the DMA
       wait transitively: ACT is in-order and every relu is gated on PE
       progress whose matmuls waited on that W DMA.
    2. The end-of-kernel quiesce drain waits on every proc lane, but the
       kernel is one dependency chain ending in the output-store DMA:
       "store landed" implies everything else.
    """
    insts = [i for blk in nc.m.functions[0].blocks for i in blk.instructions]
    for inst in insts:
        ty = type(inst).__name__
        si = inst.sync_info
        if si is None or len(si.on_wait) <= 1:
            continue
        if ty == "InstDMACopy":
            waits = list(si.on_wait)
            engine = [
                w
                for w in waits
                if w.ant_name.split("_")[0] in ("PE", "Activation", "DVE", "Pool", "SP")
            ]
            rest = [w for w in waits if w not in engine]
            dma_lanes = [w for w in rest if w.ant_name.startswith("DMA")]
            if len(engine) == 1 and len(dma_lanes) == len(rest):
                si.on_wait = engine
                continue
        if ty in ("InstDMACopy", "InstMatmult", "InstActivation"):
            raise RuntimeError(
                f"{inst.name} ({ty}) still has {len(si.on_wait)} waits: {si}"
            )

    store = [i for i in insts if type(i).__name__ == "InstDMACopy"][-1]
    assert store.sync_info and len(store.sync_info.on_update) == 1
    lane = store.sync_info.on_update[0].ant_name
    cum = 0
    for i in insts:
        if i.sync_info:
            cum += sum(
                u.update_value for u in i.sync_info.on_update if u.ant_name == lane
            )
    for inst in insts:
        if type(inst).__name__ != "InstDrain":
            continue
        si = inst.sync_info
        if si is None or len(si.on_wait) <= 1:
            continue
        keep = [w for w in si.on_wait if w.ant_name == lane and w.wait_value == cum]
        assert keep, f"drain {inst.name} lacks the store-lane wait (cum={cum}): {si}"
        si.on_wait = keep[:1]


def _strip_ceremony(nc):
    """Remove the all-engine butterfly barriers that bracket the kernel.

    Every data dependency is carried by absolute-valued semaphore waits from
    a zeroed sem file, so engines may enter their streams unaligned.
    """
    blocks = nc.m.functions[0].blocks
    b0 = blocks[0]
    drop = [
        n
        for n, i in enumerate(b0.instructions)
        if type(i).__name__ in ("InstDrain", "InstEventSemaphore")
    ]
    for n in reversed(drop):
        del b0.instructions[n]

    end = blocks[-1]
    isa_idx = [
        n for n, i in enumerate(end.instructions) if type(i).__name__ == "InstISA"
    ]
    if isa_idx:
        for n in range(len(end.instructions) - 1, isa_idx[-1], -1):
            del end.instructions[n]


def kernel(**inputs):
    ec = np.asarray(inputs["ec_activations"], dtype=np.float32)
    place = np.asarray(inputs["place_cells"], dtype=np.float32)
    grids = [np.asarray(inputs[f"grid{i}"], dtype=np.float32) for i in range(3)]
    W_ec = np.asarray(inputs["W_ec"], dtype=np.float32)
    W_mec = [np.asarray(inputs[f"W_mec{i}"], dtype=np.float32) for i in range(3)]

    X = np.concatenate(grids, axis=1)                                   # [1, 7168]
    A = np.concatenate([ec, np.broadcast_to(X, (B, X.shape[1]))], 1)    # [4, 11264]
    A = A / W_SCALE              # fold the W descale into A
    # swizzle A.T into [p, (k b)] chunk-major layout
    aT_sw = np.ascontiguousarray(
        A.T.reshape(KC, P, B).transpose(1, 0, 2)
    ).reshape(P, KC * B).astype(np.float16)

    W_all = np.concatenate([W_ec] + W_mec, axis=0)                      # [11264, 8192]
    Wq = (W_all * W_SCALE).astype(ml_dtypes.float8_e3m4)

    key = "nc_v2_" + str(CONFIG["strip_ceremony"])
    nc = _CACHE.get(key)
    if nc is None:
        nc = _CACHE[key] = _build()

    in_maps = []
    for c in range(N_CORES):
        cols = slice(SHARD * c, SHARD * (c + 1))
        # [t*128+p, k*128+j] = Wq[k*128+p, t*128+j]
        w_sw = np.ascontiguousarray(
            Wq[:, cols].reshape(KC, P, T, P).transpose(2, 1, 0, 3)
        ).reshape(T * P, KC * P)
        pl_sw = np.ascontiguousarray(
            place[0, cols].reshape(T, P).T
        ).astype(np.float32)                                            # [128, 8]
        a_host = np.concatenate([aT_sw, pl_sw.view(np.float16)], axis=1)
        in_maps.append({"a": a_host, "w": w_sw})
    res = run_bass_kernel_spmd(
        nc, in_maps, core_ids=list(range(N_CORES)), trace=CONFIG["trace"]
    )
    _CACHE["last_results"] = res
    outs = []
    for c in range(N_CORES):
        o = np.asarray(res.results[c]["out"])                           # [128, 8*4]
        outs.append(o.reshape(P, T, B).transpose(2, 1, 0).reshape(B, SHARD))
    return np.concatenate(outs, axis=1)
